# revision 26
# baseline (speedup 1.0000x reference)
"""Adaptive top-k MoE router on 8 TRN2 NeuronCores.

Data-parallel over tokens: each core routes T/8 = 2048 tokens.
Per core: weight-stationary bf16 matmul (fp32 PSUM accum) -> bf16 logits
-> PE transpose to token-major -> ACT exp/ln + DVE softmax/entropy/top-8
-> adaptive-k mask + renormalize -> int32/bf16 outputs.

Host side: shards hidden along T, pre-packed into a (t_tile, chunk_quad,
partition, contiguous-4KB) layout so every device DMA is a dense 512KB
transfer; replicates a rearranged weight; reassembles the full outputs.
"""

import numpy as np
import ml_dtypes
from contextlib import ExitStack

P = 128            # SBUF partitions
E = 64             # experts
H = 4096           # hidden dim
T_FULL = 16384     # total tokens
N_CORES = 8
T_LOC = T_FULL // N_CORES   # 2048 tokens per core
NCH = H // P                # 32 contraction chunks
TT = 512                    # token tile (matmul free dim)
NT = T_LOC // TT            # 4 token tiles
G = TT // P                 # 4 groups of 128 tokens per tile
NG = T_LOC // P             # 16 groups per core
QJ = 4                      # chunks per DMA quad
NQ = NCH // QJ              # 8 quads
N_WARM = 16                 # PE warm-up matmuls

BF16 = ml_dtypes.bfloat16

_CACHE = {}


def _pin_act_tables():
    """Make every ACT table set except natural_log_exp_and_others look like
    it contains none of {Copy, Exp, Ln}, so the table chooser emits a single
    ACT_TABLE_LOAD instead of thrashing between the exp and ln sets.
    Dict order/length is preserved (index == act_func_set_id)."""
    import concourse.hw_specs as hw_specs
    import concourse.bacc as bacc_mod
    import concourse.mybir as mybir

    if _CACHE.get("act_patched"):
        return
    orig = hw_specs.get_activation_tables
    mine = {mybir.ActivationFunctionType.Copy,
            mybir.ActivationFunctionType.Exp,
            mybir.ActivationFunctionType.Ln}

    def patched(module_arch):
        tables = dict(orig(module_arch))
        out = {}
        for name, funcs in tables.items():
            if name == "natural_log_exp_and_others":
                out[name] = funcs
            else:
                out[name] = funcs - mine
        return out

    hw_specs.get_activation_tables = patched
    bacc_mod.get_activation_tables = patched
    _CACHE["act_patched"] = True


def build_nc(t_loc=T_LOC):
    """Build the single-core Bass graph (same NEFF runs SPMD on all 8 cores)."""
    import os
    import concourse.bass as bass
    import concourse.bacc as bacc
    import concourse.mybir as mybir
    from concourse import tile
    from concourse.masks import make_identity

    stage = os.environ.get("K_STAGE", "full")   # mm | tr | sm | full
    nt_build = int(os.environ.get("K_NT", "0"))  # 0 = all tiles
    _pin_act_tables()

    nt = t_loc // TT
    ng = t_loc // P

    f32 = mybir.dt.float32
    bf16 = mybir.dt.bfloat16
    i32 = mybir.dt.int32
    u32 = mybir.dt.uint32
    AX = mybir.AxisListType.X
    OP = mybir.AluOpType
    AF = mybir.ActivationFunctionType

    nc = bacc.Bacc("TRN2", target_bir_lowering=False, debug=False)

    # ht[t, q, p, j*TT + u] = hiddenT[(4q+j)*128 + p, t*TT + u]
    ht = nc.declare_dram_parameter("ht", [nt, NQ, P, QJ * TT], bf16,
                                   isOutput=False)
    wt = nc.declare_dram_parameter("wt", [P, NCH * E], bf16, isOutput=False)
    oi = nc.declare_dram_parameter("oi", [P, ng, 4], i32, isOutput=True)
    ow = nc.declare_dram_parameter("ow", [P, ng, 4], bf16, isOutput=True)
    ok = nc.declare_dram_parameter("ok", [P, ng], i32, isOutput=True)

    with tile.TileContext(nc) as tc, ExitStack() as ctx:
        const = ctx.enter_context(tc.tile_pool(name="const", bufs=1))
        htp = ctx.enter_context(tc.tile_pool(name="htp", bufs=6))
        mmps = ctx.enter_context(
            tc.tile_pool(name="mmps", bufs=4, space=bass.MemorySpace.PSUM))
        trps = ctx.enter_context(
            tc.tile_pool(name="trps", bufs=1, space=bass.MemorySpace.PSUM))
        work = ctx.enter_context(tc.tile_pool(name="work", bufs=2))
        outp = ctx.enter_context(tc.tile_pool(name="outp", bufs=1))

        wt_s = const.tile([P, NCH * E], bf16)
        nc.scalar.dma_start(wt_s[:], wt.ap())
        identb = const.tile([E, E], bf16)
        make_identity(nc, identb[:])
        eps = const.tile([P, 1], f32)
        nc.vector.memset(eps[:], 1e-9)
        oi_s = outp.tile([P, ng, 4], i32)
        ow_s = outp.tile([P, ng, 4], bf16)
        ok_s = outp.tile([P, ng], i32)
        if stage != "full":
            nc.gpsimd.memset(oi_s[:], 0)
            nc.gpsimd.memset(ow_s[:], 0.0)
            nc.gpsimd.memset(ok_s[:], 0)

        for t in range(nt_build or nt):
            # ---- router matmuls: even chunks in PE columns 0-63, odd in
            # 64-127 (col tiling) — two matmuls in flight, halving PE time
            # and letting LDWEIGHTS overlap. Half-sums land on PSUM
            # partitions 0-63 / 64-127.
            lg_ps = mmps.tile([P, TT], f32, tag="lgps")
            for q in range(NQ):
                htile = htp.tile([P, QJ, TT], bf16, tag="ht")
                nc.sync.dma_start(htile[:], ht.ap()[t, q])
                for j in range(QJ):
                    c = QJ * q + j
                    half = c % 2
                    # skip_group_check: the sim's PSUM-group tracker is not
                    # partition-base aware; the two halves are physically
                    # disjoint partition ranges of the bank.
                    nc.tensor.matmul(
                        lg_ps[half * E:(half + 1) * E, :],
                        wt_s[:, bass.ts(c, E)], htile[:, j, :],
                        start=(c < 2), stop=(c >= NCH - 2),
                        tile_position=(0, half * E),
                        skip_group_check=True)

            # ---- merge the two half-sums: copy PSUM out, then a cross-
            # partition SBUF->SBUF accumulate-DMA adds partitions 64-127
            # onto 0-63 in exact fp32.
            sb_all = work.tile([P, TT], f32, tag="sball")
            nc.scalar.copy(sb_all[:], lg_ps[:])
            if stage == "mm":
                continue
            nc.gpsimd.dma_start(sb_all[0:E, :], sb_all[E:P, :],
                                accum_op=OP.add)
            # single fp32 -> bf16 rounding (the reference's einsum output)
            lgb = work.tile([E, TT], bf16, tag="lgb")
            nc.scalar.copy(lgb[:], sb_all[0:E, :])

            # ---- PE transpose to token-major: (128 tok, G, 64 exp)
            tr_ps = trps.tile([P, G, E], bf16, tag="trps")
            for g in range(G):
                nc.tensor.transpose(tr_ps[:, g, :], lgb[:, bass.ts(g, P)],
                                    identb[:])
            lgt = work.tile([P, G, E], bf16, tag="lgt")
            nc.vector.tensor_copy(lgt[:], tr_ps[:])
            if stage == "tr":
                continue

            # ---- softmax (fp32)
            mneg = work.tile([P, G], f32, tag="mneg")
            nc.vector.reduce_max(mneg[:], lgt[:], axis=AX, negate=True)
            pe_t = work.tile([P, G, E], f32, tag="pe")
            zs = work.tile([P, G], f32, tag="zs")
            for g in range(G):
                nc.scalar.activation(
                    pe_t[:, g, :], lgt[:, g, :], AF.Exp,
                    bias=mneg[:, g:g + 1], scale=1.0,
                    accum_out=zs[:, g:g + 1])
            rz = work.tile([P, G, 1], f32, tag="rz")
            nc.vector.reciprocal(rz[:, :, 0], zs[:])
            pn = work.tile([P, G, E], f32, tag="pn")
            a_bc, b_bc = bass.broadcast_tensor_aps(pe_t[:], rz[:])
            nc.vector.tensor_tensor(pn[:], a_bc, b_bc, OP.mult)

            # ---- entropy_neg = sum p*log(p + 1e-9)
            lq = work.tile([P, G, E], f32, tag="lq")
            nc.scalar.activation(lq[:], pn[:], AF.Ln, bias=eps[:], scale=1.0)
            pl = work.tile([P, G, E], f32, tag="pl")
            nc.vector.tensor_tensor(pl[:], pn[:], lq[:], OP.mult)
            entn = work.tile([P, G], f32, tag="entn")
            nc.vector.reduce_sum(entn[:], pl[:], axis=AX)

            # ---- adaptive k:  entropy<0.3 -> 1, >1.5 -> 4, else 2
            # entn = -entropy:  k>=2 iff entn <= -0.3 ;  k==4 iff entn < -1.5
            m2 = work.tile([P, G, 1], f32, tag="m2")
            nc.vector.tensor_scalar(m2[:, :, 0], entn[:], -0.3, None, OP.is_le)
            m4 = work.tile([P, G, 1], f32, tag="m4")
            nc.vector.tensor_scalar(m4[:, :, 0], entn[:], -1.5, None, OP.is_lt)
            kf = work.tile([P, G], f32, tag="kf")
            nc.vector.scalar_tensor_tensor(
                kf[:], m4[:, :, 0], 2.0, m2[:, :, 0], OP.mult, OP.add)
            nc.vector.tensor_scalar_add(kf[:], kf[:], 1.0)
            nc.vector.tensor_copy(ok_s[:, bass.ts(t, G)], kf[:])

            if stage == "sm":
                continue
            # ---- top-8 values + indices (ties: descending value, ascending idx)
            mv = work.tile([P, G, 8], f32, tag="mv")
            mi = work.tile([P, G, 8], u32, tag="mi")
            for g in range(G):
                nc.vector.max(mv[:, g, :], pe_t[:, g, :])
                nc.vector.max_index(mi[:, g, :], mv[:, g, :], pe_t[:, g, :])

            # ---- active-slot mask (slot0: always, slot1: k>=2, slot2/3: k==4)
            act = work.tile([P, G, 4], f32, tag="act")
            nc.vector.memset(act[:, :, 0:1], 1.0)
            nc.vector.tensor_copy(act[:, :, 1:2], m2[:])
            nc.vector.tensor_copy(act[:, :, 2:3], m4[:])
            nc.vector.tensor_copy(act[:, :, 3:4], m4[:])

            # ---- masked renormalized weights
            w4 = work.tile([P, G, 4], f32, tag="w4")
            nc.vector.tensor_tensor(w4[:], mv[:, :, 0:4], act[:], OP.mult)
            ws = work.tile([P, G], f32, tag="ws")
            nc.vector.reduce_sum(ws[:], w4[:], axis=AX)
            rw = work.tile([P, G, 1], f32, tag="rw")
            nc.vector.reciprocal(rw[:, :, 0], ws[:])
            wn = work.tile([P, G, 4], f32, tag="wn")
            wa_bc, wb_bc = bass.broadcast_tensor_aps(w4[:], rw[:])
            nc.vector.tensor_tensor(wn[:], wa_bc, wb_bc, OP.mult)
            nc.vector.tensor_copy(ow_s[:, bass.ts(t, G)], wn[:])

            # ---- indices: (idx+1)*active - 1  (pads inactive slots with -1)
            idxf = work.tile([P, G, 4], f32, tag="idxf")
            nc.vector.tensor_copy(idxf[:], mi[:, :, 0:4])
            nc.vector.scalar_tensor_tensor(
                idxf[:], idxf[:], 1.0, act[:], OP.add, OP.mult)
            nc.vector.tensor_scalar_add(idxf[:], idxf[:], -1.0)
            nc.vector.tensor_copy(oi_s[:, bass.ts(t, G)], idxf[:])

        nc.scalar.dma_start(oi.ap(), oi_s[:])
        nc.scalar.dma_start(ow.ap(), ow_s[:])
        nc.scalar.dma_start(ok.ap(), ok_s[:])

    nc.finalize()
    return nc


def _get_nc():
    if "nc" not in _CACHE:
        _CACHE["nc"] = build_nc()
    return _CACHE["nc"]


def _prep_shards(hidden, weight):
    hidden = np.asarray(hidden)
    weight = np.asarray(weight)
    if hidden.dtype != BF16:
        hidden = hidden.astype(BF16)
    if weight.dtype != BF16:
        weight = weight.astype(BF16)
    # weight (E, H) -> wt[p, c*E + e] = weight[e, c*P + p]
    wt = np.ascontiguousarray(
        weight.reshape(E, NCH, P).transpose(2, 1, 0).reshape(P, NCH * E))
    # hidden (T, H) -> hiddenT (H, T), shard along tokens, pack per-DMA-dense:
    # ht[t, q, p, j*TT+u] = hiddenT[(QJ*q+j)*P + p, t*TT + u]
    ht_full = np.ascontiguousarray(hidden.T)
    in_maps = []
    for c in range(N_CORES):
        s = ht_full[:, c * T_LOC:(c + 1) * T_LOC]          # (H, T_LOC)
        s5 = s.reshape(NQ, QJ, P, NT, TT)                  # (q, j, p, t, u)
        ht_shard = np.ascontiguousarray(
            s5.transpose(3, 0, 2, 1, 4).reshape(NT, NQ, P, QJ * TT))
        in_maps.append({"ht": ht_shard, "wt": wt})
    return in_maps


def _assemble(results):
    idx_parts, w_parts, k_parts = [], [], []
    for c in range(N_CORES):
        oi = np.asarray(results[c]["oi"])            # (P, NG, 4) int32
        ow = np.asarray(results[c]["ow"])            # (P, NG, 4) bf16
        ok = np.asarray(results[c]["ok"])            # (P, NG)   int32
        # token = g*128 + p  ->  [g, p, s]
        idx_parts.append(oi.transpose(1, 0, 2).reshape(T_LOC, 4))
        w_parts.append(ow.transpose(1, 0, 2).reshape(T_LOC, 4))
        k_parts.append(ok.transpose(1, 0).reshape(T_LOC))
    indices = np.concatenate(idx_parts, axis=0).astype(np.int32)
    weights = np.concatenate(w_parts, axis=0)
    if weights.dtype != BF16:
        weights = weights.view(BF16) if weights.dtype.itemsize == 2 \
            else weights.astype(BF16)
    k = np.concatenate(k_parts, axis=0).astype(np.int32)
    return indices, weights, k


def kernel(hidden, weight):
    from concourse.bass_utils import run_bass_kernel_spmd

    nc = _get_nc()
    in_maps = _prep_shards(hidden, weight)
    res = run_bass_kernel_spmd(nc, in_maps, core_ids=list(range(N_CORES)))
    return _assemble(res.results)


# revision 31
# speedup vs baseline: 1.0149x; 1.0149x over previous
"""Adaptive top-k MoE router on 8 TRN2 NeuronCores.

Data-parallel over tokens: each core routes T/8 = 2048 tokens.
Per core: weight-stationary bf16 matmul (fp32 PSUM accum) -> bf16 logits
-> PE transpose to token-major -> ACT exp/ln + DVE softmax/entropy/top-8
-> adaptive-k mask + renormalize -> int32/bf16 outputs.

Host side: shards hidden along T, pre-packed into a (t_tile, chunk_quad,
partition, contiguous-4KB) layout so every device DMA is a dense 512KB
transfer; replicates a rearranged weight; reassembles the full outputs.
"""

import numpy as np
import ml_dtypes
from contextlib import ExitStack

P = 128            # SBUF partitions
E = 64             # experts
H = 4096           # hidden dim
T_FULL = 16384     # total tokens
N_CORES = 8
T_LOC = T_FULL // N_CORES   # 2048 tokens per core
NCH = H // P                # 32 contraction chunks
TT = 512                    # token tile (matmul free dim)
NT = T_LOC // TT            # 4 token tiles
G = TT // P                 # 4 groups of 128 tokens per tile
NG = T_LOC // P             # 16 groups per core
QJ = 4                      # chunks per DMA quad
NQ = NCH // QJ              # 8 quads
N_WARM = 16                 # PE warm-up matmuls

BF16 = ml_dtypes.bfloat16

_CACHE = {}


def _pin_act_tables():
    """Make every ACT table set except natural_log_exp_and_others look like
    it contains none of {Copy, Exp, Ln}, so the table chooser emits a single
    ACT_TABLE_LOAD instead of thrashing between the exp and ln sets.
    Dict order/length is preserved (index == act_func_set_id)."""
    import concourse.hw_specs as hw_specs
    import concourse.bacc as bacc_mod
    import concourse.mybir as mybir

    if _CACHE.get("act_patched"):
        return
    orig = hw_specs.get_activation_tables
    mine = {mybir.ActivationFunctionType.Copy,
            mybir.ActivationFunctionType.Exp,
            mybir.ActivationFunctionType.Ln}

    def patched(module_arch):
        tables = dict(orig(module_arch))
        out = {}
        for name, funcs in tables.items():
            if name == "natural_log_exp_and_others":
                out[name] = funcs
            else:
                out[name] = funcs - mine
        return out

    hw_specs.get_activation_tables = patched
    bacc_mod.get_activation_tables = patched
    _CACHE["act_patched"] = True


def build_nc(t_loc=T_LOC):
    """Build the single-core Bass graph (same NEFF runs SPMD on all 8 cores)."""
    import os
    import concourse.bass as bass
    import concourse.bacc as bacc
    import concourse.mybir as mybir
    from concourse import tile
    from concourse.masks import make_identity

    stage = os.environ.get("K_STAGE", "full")   # mm | tr | sm | full
    nt_build = int(os.environ.get("K_NT", "0"))  # 0 = all tiles
    _pin_act_tables()

    nt = t_loc // TT
    ng = t_loc // P

    f32 = mybir.dt.float32
    bf16 = mybir.dt.bfloat16
    i32 = mybir.dt.int32
    u32 = mybir.dt.uint32
    AX = mybir.AxisListType.X
    OP = mybir.AluOpType
    AF = mybir.ActivationFunctionType

    nc = bacc.Bacc("TRN2", target_bir_lowering=False, debug=False)

    # ht[t, q, p, j*TT + u] = hiddenT[(4q+j)*128 + p, t*TT + u]
    ht = nc.declare_dram_parameter("ht", [nt, NQ, P, QJ * TT], bf16,
                                   isOutput=False)
    wt = nc.declare_dram_parameter("wt", [P, NCH * E], bf16, isOutput=False)
    oi = nc.declare_dram_parameter("oi", [P, ng, 4], i32, isOutput=True)
    ow = nc.declare_dram_parameter("ow", [P, ng, 4], bf16, isOutput=True)
    ok = nc.declare_dram_parameter("ok", [P, ng], i32, isOutput=True)

    with tile.TileContext(nc) as tc, ExitStack() as ctx:
        const = ctx.enter_context(tc.tile_pool(name="const", bufs=1))
        htp = ctx.enter_context(tc.tile_pool(name="htp", bufs=6))
        mmps = ctx.enter_context(
            tc.tile_pool(name="mmps", bufs=4, space=bass.MemorySpace.PSUM))
        mgps = ctx.enter_context(
            tc.tile_pool(name="mgps", bufs=2, space=bass.MemorySpace.PSUM))
        trps = ctx.enter_context(
            tc.tile_pool(name="trps", bufs=2, space=bass.MemorySpace.PSUM))
        work = ctx.enter_context(tc.tile_pool(name="work", bufs=2))
        outp = ctx.enter_context(tc.tile_pool(name="outp", bufs=1))

        wt_s = const.tile([P, NCH * E], bf16)
        nc.scalar.dma_start(wt_s[:], wt.ap())
        identb = const.tile([E, E], bf16)
        make_identity(nc, identb[:])
        identf = const.tile([P, P], f32)
        make_identity(nc, identf[:])
        eps = const.tile([P, 1], f32)
        nc.vector.memset(eps[:], 1e-9)
        oi_s = outp.tile([P, ng, 4], i32)
        ow_s = outp.tile([P, ng, 4], bf16)
        ok_s = outp.tile([P, ng], i32)
        if stage != "full":
            nc.gpsimd.memset(oi_s[:], 0)
            nc.gpsimd.memset(ow_s[:], 0.0)
            nc.gpsimd.memset(ok_s[:], 0)

        def emit_mm(t):
            # ---- router matmuls: even chunks in PE columns 0-63, odd in
            # 64-127 (col tiling) — two matmuls in flight, halving PE time
            # and letting LDWEIGHTS overlap. Half-sums land on PSUM
            # partitions 0-63 / 64-127.
            lg_ps = mmps.tile([P, TT], f32, tag="lgps")
            for q in range(NQ):
                htile = htp.tile([P, QJ, TT], bf16, tag="ht")
                nc.sync.dma_start(htile[:], ht.ap()[t, q])
                for j in range(QJ):
                    c = QJ * q + j
                    half = c % 2
                    # skip_group_check: the sim's PSUM-group tracker is not
                    # partition-base aware; the two halves are physically
                    # disjoint partition ranges of the bank.
                    nc.tensor.matmul(
                        lg_ps[half * E:(half + 1) * E, :],
                        wt_s[:, bass.ts(c, E)], htile[:, j, :],
                        start=(c < 2), stop=(c >= NCH - 2),
                        tile_position=(0, half * E),
                        skip_group_check=True)
            return lg_ps

        def emit_post(t, lg_ps):
            # ---- merge the two half-sums: copy PSUM out, then a cross-
            # partition SBUF->SBUF accumulate-DMA adds partitions 64-127
            # onto 0-63 in exact fp32.
            sb_all = work.tile([P, TT], f32, tag="sball")
            nc.scalar.copy(sb_all[:], lg_ps[:])
            nc.gpsimd.dma_start(sb_all[0:E, :], sb_all[E:P, :],
                                accum_op=OP.add)
            # single fp32 -> bf16 rounding (the reference's einsum output)
            lgb = work.tile([E, TT], bf16, tag="lgb")
            nc.scalar.copy(lgb[:], sb_all[0:E, :])

            # ---- PE transpose to token-major: (128 tok, G, 64 exp)
            tr_ps = trps.tile([P, G, E], bf16, tag="trps")
            for g in range(G):
                nc.tensor.transpose(tr_ps[:, g, :], lgb[:, bass.ts(g, P)],
                                    identb[:])
            lgt = work.tile([P, G, E], bf16, tag="lgt")
            nc.vector.tensor_copy(lgt[:], tr_ps[:])
            if stage == "tr":
                return

            # ---- softmax (fp32)
            mneg = work.tile([P, G], f32, tag="mneg")
            nc.vector.reduce_max(mneg[:], lgt[:], axis=AX, negate=True)
            pe_t = work.tile([P, G, E], f32, tag="pe")
            zs = work.tile([P, G], f32, tag="zs")
            for g in range(G):
                nc.scalar.activation(
                    pe_t[:, g, :], lgt[:, g, :], AF.Exp,
                    bias=mneg[:, g:g + 1], scale=1.0,
                    accum_out=zs[:, g:g + 1])
            rz = work.tile([P, G, 1], f32, tag="rz")
            nc.vector.reciprocal(rz[:, :, 0], zs[:])
            pn = work.tile([P, G, E], f32, tag="pn")
            a_bc, b_bc = bass.broadcast_tensor_aps(pe_t[:], rz[:])
            nc.vector.tensor_tensor(pn[:], a_bc, b_bc, OP.mult)

            # ---- entropy_neg = sum p*log(p + 1e-9)
            lq = work.tile([P, G, E], f32, tag="lq")
            nc.scalar.activation(lq[:], pn[:], AF.Ln, bias=eps[:], scale=1.0)
            pl = work.tile([P, G, E], f32, tag="pl")
            nc.vector.tensor_tensor(pl[:], pn[:], lq[:], OP.mult)
            entn = work.tile([P, G], f32, tag="entn")
            nc.vector.reduce_sum(entn[:], pl[:], axis=AX)

            # ---- adaptive k:  entropy<0.3 -> 1, >1.5 -> 4, else 2
            # entn = -entropy:  k>=2 iff entn <= -0.3 ;  k==4 iff entn < -1.5
            m2 = work.tile([P, G, 1], f32, tag="m2")
            nc.vector.tensor_scalar(m2[:, :, 0], entn[:], -0.3, None, OP.is_le)
            m4 = work.tile([P, G, 1], f32, tag="m4")
            nc.vector.tensor_scalar(m4[:, :, 0], entn[:], -1.5, None, OP.is_lt)
            kf = work.tile([P, G], f32, tag="kf")
            nc.vector.scalar_tensor_tensor(
                kf[:], m4[:, :, 0], 2.0, m2[:, :, 0], OP.mult, OP.add)
            nc.vector.tensor_scalar_add(kf[:], kf[:], 1.0)
            nc.vector.tensor_copy(ok_s[:, bass.ts(t, G)], kf[:])

            if stage == "sm":
                return
            # ---- top-8 values + indices (ties: descending value, ascending idx)
            mv = work.tile([P, G, 8], f32, tag="mv")
            mi = work.tile([P, G, 8], u32, tag="mi")
            for g in range(G):
                nc.vector.max(mv[:, g, :], pe_t[:, g, :])
                nc.vector.max_index(mi[:, g, :], mv[:, g, :], pe_t[:, g, :])

            # ---- active-slot mask (slot0: always, slot1: k>=2, slot2/3: k==4)
            act = work.tile([P, G, 4], f32, tag="act")
            nc.vector.memset(act[:, :, 0:1], 1.0)
            nc.vector.tensor_copy(act[:, :, 1:2], m2[:])
            nc.vector.tensor_copy(act[:, :, 2:3], m4[:])
            nc.vector.tensor_copy(act[:, :, 3:4], m4[:])

            # ---- masked renormalized weights
            w4 = work.tile([P, G, 4], f32, tag="w4")
            nc.vector.tensor_tensor(w4[:], mv[:, :, 0:4], act[:], OP.mult)
            ws = work.tile([P, G], f32, tag="ws")
            nc.vector.reduce_sum(ws[:], w4[:], axis=AX)
            rw = work.tile([P, G, 1], f32, tag="rw")
            nc.vector.reciprocal(rw[:, :, 0], ws[:])
            wn = work.tile([P, G, 4], f32, tag="wn")
            wa_bc, wb_bc = bass.broadcast_tensor_aps(w4[:], rw[:])
            nc.vector.tensor_tensor(wn[:], wa_bc, wb_bc, OP.mult)
            nc.vector.tensor_copy(ow_s[:, bass.ts(t, G)], wn[:])

            # ---- indices: (idx+1)*active - 1  (pads inactive slots with -1)
            idxf = work.tile([P, G, 4], f32, tag="idxf")
            nc.vector.tensor_copy(idxf[:], mi[:, :, 0:4])
            nc.vector.scalar_tensor_tensor(
                idxf[:], idxf[:], 1.0, act[:], OP.add, OP.mult)
            nc.vector.tensor_scalar_add(idxf[:], idxf[:], -1.0)
            nc.vector.tensor_copy(oi_s[:, bass.ts(t, G)], idxf[:])

        # software pipeline: tile t's matmuls stream while tile t-1's
        # transposes/softmax drain — only the last tile's post is a tail
        ntl = nt_build or nt
        prev = None
        for t in range(ntl):
            lg = emit_mm(t)
            if prev is not None:
                emit_post(t - 1, prev)
            prev = lg
        emit_post(ntl - 1, prev)

        nc.scalar.dma_start(oi.ap(), oi_s[:])
        nc.scalar.dma_start(ow.ap(), ow_s[:])
        nc.scalar.dma_start(ok.ap(), ok_s[:])

    nc.finalize()
    return nc


def _get_nc():
    if "nc" not in _CACHE:
        _CACHE["nc"] = build_nc()
    return _CACHE["nc"]


def _prep_shards(hidden, weight):
    hidden = np.asarray(hidden)
    weight = np.asarray(weight)
    if hidden.dtype != BF16:
        hidden = hidden.astype(BF16)
    if weight.dtype != BF16:
        weight = weight.astype(BF16)
    # weight (E, H) -> wt[p, c*E + e] = weight[e, c*P + p]
    wt = np.ascontiguousarray(
        weight.reshape(E, NCH, P).transpose(2, 1, 0).reshape(P, NCH * E))
    # hidden (T, H) -> hiddenT (H, T), shard along tokens, pack per-DMA-dense:
    # ht[t, q, p, j*TT+u] = hiddenT[(QJ*q+j)*P + p, t*TT + u]
    ht_full = np.ascontiguousarray(hidden.T)
    in_maps = []
    for c in range(N_CORES):
        s = ht_full[:, c * T_LOC:(c + 1) * T_LOC]          # (H, T_LOC)
        s5 = s.reshape(NQ, QJ, P, NT, TT)                  # (q, j, p, t, u)
        ht_shard = np.ascontiguousarray(
            s5.transpose(3, 0, 2, 1, 4).reshape(NT, NQ, P, QJ * TT))
        in_maps.append({"ht": ht_shard, "wt": wt})
    return in_maps


def _assemble(results):
    idx_parts, w_parts, k_parts = [], [], []
    for c in range(N_CORES):
        oi = np.asarray(results[c]["oi"])            # (P, NG, 4) int32
        ow = np.asarray(results[c]["ow"])            # (P, NG, 4) bf16
        ok = np.asarray(results[c]["ok"])            # (P, NG)   int32
        # token = g*128 + p  ->  [g, p, s]
        idx_parts.append(oi.transpose(1, 0, 2).reshape(T_LOC, 4))
        w_parts.append(ow.transpose(1, 0, 2).reshape(T_LOC, 4))
        k_parts.append(ok.transpose(1, 0).reshape(T_LOC))
    indices = np.concatenate(idx_parts, axis=0).astype(np.int32)
    weights = np.concatenate(w_parts, axis=0)
    if weights.dtype != BF16:
        weights = weights.view(BF16) if weights.dtype.itemsize == 2 \
            else weights.astype(BF16)
    k = np.concatenate(k_parts, axis=0).astype(np.int32)
    return indices, weights, k


def kernel(hidden, weight):
    from concourse.bass_utils import run_bass_kernel_spmd

    nc = _get_nc()
    in_maps = _prep_shards(hidden, weight)
    res = run_bass_kernel_spmd(nc, in_maps, core_ids=list(range(N_CORES)))
    return _assemble(res.results)


# revision 32
# speedup vs baseline: 1.1228x; 1.1063x over previous
"""Adaptive top-k MoE router on 8 TRN2 NeuronCores.

Data-parallel over tokens: each core routes T/8 = 2048 tokens.
Per core: weight-stationary bf16 matmul (fp32 PSUM accum) -> bf16 logits
-> PE transpose to token-major -> ACT exp/ln + DVE softmax/entropy/top-8
-> adaptive-k mask + renormalize -> int32/bf16 outputs.

Host side: shards hidden along T, pre-packed into a (t_tile, chunk_quad,
partition, contiguous-4KB) layout so every device DMA is a dense 512KB
transfer; replicates a rearranged weight; reassembles the full outputs.
"""

import numpy as np
import ml_dtypes
from contextlib import ExitStack

P = 128            # SBUF partitions
E = 64             # experts
H = 4096           # hidden dim
T_FULL = 16384     # total tokens
N_CORES = 8
T_LOC = T_FULL // N_CORES   # 2048 tokens per core
NCH = H // P                # 32 contraction chunks
TT = 512                    # token tile (matmul free dim)
NT = T_LOC // TT            # 4 token tiles
G = TT // P                 # 4 groups of 128 tokens per tile
NG = T_LOC // P             # 16 groups per core
QJ = 4                      # chunks per DMA quad
NQ = NCH // QJ              # 8 quads
N_WARM = 16                 # PE warm-up matmuls

BF16 = ml_dtypes.bfloat16

_CACHE = {}


def _pin_act_tables():
    """Make every ACT table set except natural_log_exp_and_others look like
    it contains none of {Copy, Exp, Ln}, so the table chooser emits a single
    ACT_TABLE_LOAD instead of thrashing between the exp and ln sets.
    Dict order/length is preserved (index == act_func_set_id)."""
    import concourse.hw_specs as hw_specs
    import concourse.bacc as bacc_mod
    import concourse.mybir as mybir

    if _CACHE.get("act_patched"):
        return
    orig = hw_specs.get_activation_tables
    mine = {mybir.ActivationFunctionType.Copy,
            mybir.ActivationFunctionType.Exp,
            mybir.ActivationFunctionType.Ln}

    def patched(module_arch):
        tables = dict(orig(module_arch))
        out = {}
        for name, funcs in tables.items():
            if name == "natural_log_exp_and_others":
                out[name] = funcs
            else:
                out[name] = funcs - mine
        return out

    hw_specs.get_activation_tables = patched
    bacc_mod.get_activation_tables = patched
    _CACHE["act_patched"] = True


def build_nc(t_loc=T_LOC):
    """Build the single-core Bass graph (same NEFF runs SPMD on all 8 cores)."""
    import os
    import concourse.bass as bass
    import concourse.bacc as bacc
    import concourse.mybir as mybir
    from concourse import tile
    from concourse.masks import make_identity
    from concourse.tile_rust import add_dep_helper

    stage = os.environ.get("K_STAGE", "full")   # mm | tr | sm | full
    nt_build = int(os.environ.get("K_NT", "0"))  # 0 = all tiles
    _pin_act_tables()

    nt = t_loc // TT
    ng = t_loc // P

    f32 = mybir.dt.float32
    bf16 = mybir.dt.bfloat16
    i32 = mybir.dt.int32
    u32 = mybir.dt.uint32
    AX = mybir.AxisListType.X
    OP = mybir.AluOpType
    AF = mybir.ActivationFunctionType

    nc = bacc.Bacc("TRN2", target_bir_lowering=False, debug=False)

    # ht[t, q, p, j*TT + u] = hiddenT[(4q+j)*128 + p, t*TT + u]
    ht = nc.declare_dram_parameter("ht", [nt, NQ, P, QJ * TT], bf16,
                                   isOutput=False)
    wt = nc.declare_dram_parameter("wt", [P, NCH * E], bf16, isOutput=False)
    oi = nc.declare_dram_parameter("oi", [P, ng, 4], i32, isOutput=True)
    ow = nc.declare_dram_parameter("ow", [P, ng, 4], bf16, isOutput=True)
    ok = nc.declare_dram_parameter("ok", [P, ng], i32, isOutput=True)

    with tile.TileContext(nc) as tc, ExitStack() as ctx:
        const = ctx.enter_context(tc.tile_pool(name="const", bufs=1))
        htp = ctx.enter_context(tc.tile_pool(name="htp", bufs=6))
        mmps = ctx.enter_context(
            tc.tile_pool(name="mmps", bufs=4, space=bass.MemorySpace.PSUM))
        mgps = ctx.enter_context(
            tc.tile_pool(name="mgps", bufs=2, space=bass.MemorySpace.PSUM))
        trps = ctx.enter_context(
            tc.tile_pool(name="trps", bufs=2, space=bass.MemorySpace.PSUM))
        work = ctx.enter_context(tc.tile_pool(name="work", bufs=2))
        outp = ctx.enter_context(tc.tile_pool(name="outp", bufs=1))

        wt_s = const.tile([P, NCH * E], bf16)
        nc.scalar.dma_start(wt_s[:], wt.ap())
        identb = const.tile([E, E], bf16)
        make_identity(nc, identb[:])
        identf = const.tile([P, P], f32)
        make_identity(nc, identf[:])
        eps = const.tile([P, 1], f32)
        nc.vector.memset(eps[:], 1e-9)
        oi_s = outp.tile([P, ng, 4], i32)
        ow_s = outp.tile([P, ng, 4], bf16)
        ok_s = outp.tile([P, ng], i32)
        if stage != "full":
            nc.gpsimd.memset(oi_s[:], 0)
            nc.gpsimd.memset(ow_s[:], 0.0)
            nc.gpsimd.memset(ok_s[:], 0)

        def emit_mm(t):
            # ---- router matmuls: even chunks in PE columns 0-63, odd in
            # 64-127 (col tiling) — two matmuls in flight, halving PE time
            # and letting LDWEIGHTS overlap. Half-sums land on PSUM
            # partitions 0-63 / 64-127.
            lg_ps = mmps.tile([P, TT], f32, tag="lgps")
            mm = None
            for q in range(NQ):
                htile = htp.tile([P, QJ, TT], bf16, tag="ht")
                nc.sync.dma_start(htile[:], ht.ap()[t, q])
                for j in range(QJ):
                    c = QJ * q + j
                    half = c % 2
                    # skip_group_check: the sim's PSUM-group tracker is not
                    # partition-base aware; the two halves are physically
                    # disjoint partition ranges of the bank.
                    mm = nc.tensor.matmul(
                        lg_ps[half * E:(half + 1) * E, :],
                        wt_s[:, bass.ts(c, E)], htile[:, j, :],
                        start=(c < 2), stop=(c >= NCH - 2),
                        tile_position=(0, half * E),
                        skip_group_check=True)
            return lg_ps, mm

        def emit_post(t, lg_ps, anchor):
            # ---- merge the two half-sums: copy PSUM out, then a cross-
            # partition SBUF->SBUF accumulate-DMA adds partitions 64-127
            # onto 0-63 in exact fp32.
            sb_all = work.tile([P, TT], f32, tag="sball")
            nc.scalar.copy(sb_all[:], lg_ps[:])
            nc.gpsimd.dma_start(sb_all[0:E, :], sb_all[E:P, :],
                                accum_op=OP.add)
            # single fp32 -> bf16 rounding (the reference's einsum output)
            lgb = work.tile([E, TT], bf16, tag="lgb")
            nc.scalar.copy(lgb[:], sb_all[0:E, :])

            # ---- PE transpose to token-major: (128 tok, G, 64 exp)
            tr_ps = trps.tile([P, G, E], bf16, tag="trps")
            for g in range(G):
                tr = nc.tensor.transpose(tr_ps[:, g, :], lgb[:, bass.ts(g, P)],
                                         identb[:])
                if anchor is not None:
                    # scheduling-only: keep this tile's transposes after the
                    # NEXT tile's final matmul so the PE never head-of-line
                    # stalls on the (slow) SWDGE merge chain
                    add_dep_helper(tr.ins, anchor.ins, sync=False,
                                   reason="tr after next tile's matmuls")
            lgt = work.tile([P, G, E], bf16, tag="lgt")
            nc.vector.tensor_copy(lgt[:], tr_ps[:])
            if stage == "tr":
                return

            # ---- softmax (fp32)
            mneg = work.tile([P, G], f32, tag="mneg")
            nc.vector.reduce_max(mneg[:], lgt[:], axis=AX, negate=True)
            pe_t = work.tile([P, G, E], f32, tag="pe")
            zs = work.tile([P, G], f32, tag="zs")
            for g in range(G):
                nc.scalar.activation(
                    pe_t[:, g, :], lgt[:, g, :], AF.Exp,
                    bias=mneg[:, g:g + 1], scale=1.0,
                    accum_out=zs[:, g:g + 1])
            rz = work.tile([P, G, 1], f32, tag="rz")
            nc.vector.reciprocal(rz[:, :, 0], zs[:])
            pn = work.tile([P, G, E], f32, tag="pn")
            a_bc, b_bc = bass.broadcast_tensor_aps(pe_t[:], rz[:])
            nc.vector.tensor_tensor(pn[:], a_bc, b_bc, OP.mult)

            # ---- entropy_neg = sum p*log(p + 1e-9)
            lq = work.tile([P, G, E], f32, tag="lq")
            nc.scalar.activation(lq[:], pn[:], AF.Ln, bias=eps[:], scale=1.0)
            pl = work.tile([P, G, E], f32, tag="pl")
            nc.vector.tensor_tensor(pl[:], pn[:], lq[:], OP.mult)
            entn = work.tile([P, G], f32, tag="entn")
            nc.vector.reduce_sum(entn[:], pl[:], axis=AX)

            # ---- adaptive k:  entropy<0.3 -> 1, >1.5 -> 4, else 2
            # entn = -entropy:  k>=2 iff entn <= -0.3 ;  k==4 iff entn < -1.5
            m2 = work.tile([P, G, 1], f32, tag="m2")
            nc.vector.tensor_scalar(m2[:, :, 0], entn[:], -0.3, None, OP.is_le)
            m4 = work.tile([P, G, 1], f32, tag="m4")
            nc.vector.tensor_scalar(m4[:, :, 0], entn[:], -1.5, None, OP.is_lt)
            kf = work.tile([P, G], f32, tag="kf")
            nc.vector.scalar_tensor_tensor(
                kf[:], m4[:, :, 0], 2.0, m2[:, :, 0], OP.mult, OP.add)
            nc.vector.tensor_scalar_add(kf[:], kf[:], 1.0)
            nc.vector.tensor_copy(ok_s[:, bass.ts(t, G)], kf[:])

            if stage == "sm":
                return
            # ---- top-8 values + indices (ties: descending value, ascending idx)
            mv = work.tile([P, G, 8], f32, tag="mv")
            mi = work.tile([P, G, 8], u32, tag="mi")
            for g in range(G):
                nc.vector.max(mv[:, g, :], pe_t[:, g, :])
                nc.vector.max_index(mi[:, g, :], mv[:, g, :], pe_t[:, g, :])

            # ---- active-slot mask (slot0: always, slot1: k>=2, slot2/3: k==4)
            act = work.tile([P, G, 4], f32, tag="act")
            nc.vector.memset(act[:, :, 0:1], 1.0)
            nc.vector.tensor_copy(act[:, :, 1:2], m2[:])
            nc.vector.tensor_copy(act[:, :, 2:3], m4[:])
            nc.vector.tensor_copy(act[:, :, 3:4], m4[:])

            # ---- masked renormalized weights
            w4 = work.tile([P, G, 4], f32, tag="w4")
            nc.vector.tensor_tensor(w4[:], mv[:, :, 0:4], act[:], OP.mult)
            ws = work.tile([P, G], f32, tag="ws")
            nc.vector.reduce_sum(ws[:], w4[:], axis=AX)
            rw = work.tile([P, G, 1], f32, tag="rw")
            nc.vector.reciprocal(rw[:, :, 0], ws[:])
            wn = work.tile([P, G, 4], f32, tag="wn")
            wa_bc, wb_bc = bass.broadcast_tensor_aps(w4[:], rw[:])
            nc.vector.tensor_tensor(wn[:], wa_bc, wb_bc, OP.mult)
            nc.vector.tensor_copy(ow_s[:, bass.ts(t, G)], wn[:])

            # ---- indices: (idx+1)*active - 1  (pads inactive slots with -1)
            idxf = work.tile([P, G, 4], f32, tag="idxf")
            nc.vector.tensor_copy(idxf[:], mi[:, :, 0:4])
            nc.vector.scalar_tensor_tensor(
                idxf[:], idxf[:], 1.0, act[:], OP.add, OP.mult)
            nc.vector.tensor_scalar_add(idxf[:], idxf[:], -1.0)
            nc.vector.tensor_copy(oi_s[:, bass.ts(t, G)], idxf[:])

        # software pipeline: tile t's matmuls stream while tile t-1's
        # transposes/softmax drain — only the last tile's post is a tail
        ntl = nt_build or nt
        prev = None
        for t in range(ntl):
            lg, mm = emit_mm(t)
            if prev is not None:
                emit_post(t - 1, prev, mm)
            prev = lg
        emit_post(ntl - 1, prev, None)

        nc.scalar.dma_start(oi.ap(), oi_s[:])
        nc.scalar.dma_start(ow.ap(), ow_s[:])
        nc.scalar.dma_start(ok.ap(), ok_s[:])

    nc.finalize()
    return nc


def _get_nc():
    if "nc" not in _CACHE:
        _CACHE["nc"] = build_nc()
    return _CACHE["nc"]


def _prep_shards(hidden, weight):
    hidden = np.asarray(hidden)
    weight = np.asarray(weight)
    if hidden.dtype != BF16:
        hidden = hidden.astype(BF16)
    if weight.dtype != BF16:
        weight = weight.astype(BF16)
    # weight (E, H) -> wt[p, c*E + e] = weight[e, c*P + p]
    wt = np.ascontiguousarray(
        weight.reshape(E, NCH, P).transpose(2, 1, 0).reshape(P, NCH * E))
    # hidden (T, H) -> hiddenT (H, T), shard along tokens, pack per-DMA-dense:
    # ht[t, q, p, j*TT+u] = hiddenT[(QJ*q+j)*P + p, t*TT + u]
    ht_full = np.ascontiguousarray(hidden.T)
    in_maps = []
    for c in range(N_CORES):
        s = ht_full[:, c * T_LOC:(c + 1) * T_LOC]          # (H, T_LOC)
        s5 = s.reshape(NQ, QJ, P, NT, TT)                  # (q, j, p, t, u)
        ht_shard = np.ascontiguousarray(
            s5.transpose(3, 0, 2, 1, 4).reshape(NT, NQ, P, QJ * TT))
        in_maps.append({"ht": ht_shard, "wt": wt})
    return in_maps


def _assemble(results):
    idx_parts, w_parts, k_parts = [], [], []
    for c in range(N_CORES):
        oi = np.asarray(results[c]["oi"])            # (P, NG, 4) int32
        ow = np.asarray(results[c]["ow"])            # (P, NG, 4) bf16
        ok = np.asarray(results[c]["ok"])            # (P, NG)   int32
        # token = g*128 + p  ->  [g, p, s]
        idx_parts.append(oi.transpose(1, 0, 2).reshape(T_LOC, 4))
        w_parts.append(ow.transpose(1, 0, 2).reshape(T_LOC, 4))
        k_parts.append(ok.transpose(1, 0).reshape(T_LOC))
    indices = np.concatenate(idx_parts, axis=0).astype(np.int32)
    weights = np.concatenate(w_parts, axis=0)
    if weights.dtype != BF16:
        weights = weights.view(BF16) if weights.dtype.itemsize == 2 \
            else weights.astype(BF16)
    k = np.concatenate(k_parts, axis=0).astype(np.int32)
    return indices, weights, k


def kernel(hidden, weight):
    from concourse.bass_utils import run_bass_kernel_spmd

    nc = _get_nc()
    in_maps = _prep_shards(hidden, weight)
    res = run_bass_kernel_spmd(nc, in_maps, core_ids=list(range(N_CORES)))
    return _assemble(res.results)


# revision 33
# speedup vs baseline: 1.2398x; 1.1042x over previous
"""Adaptive top-k MoE router on 8 TRN2 NeuronCores.

Data-parallel over tokens: each core routes T/8 = 2048 tokens.
Per core: weight-stationary bf16 matmul (fp32 PSUM accum) -> bf16 logits
-> PE transpose to token-major -> ACT exp/ln + DVE softmax/entropy/top-8
-> adaptive-k mask + renormalize -> int32/bf16 outputs.

Host side: shards hidden along T, pre-packed into a (t_tile, chunk_quad,
partition, contiguous-4KB) layout so every device DMA is a dense 512KB
transfer; replicates a rearranged weight; reassembles the full outputs.
"""

import numpy as np
import ml_dtypes
from contextlib import ExitStack

P = 128            # SBUF partitions
E = 64             # experts
H = 4096           # hidden dim
T_FULL = 16384     # total tokens
N_CORES = 8
T_LOC = T_FULL // N_CORES   # 2048 tokens per core
NCH = H // P                # 32 contraction chunks
TT = 512                    # token tile (matmul free dim)
NT = T_LOC // TT            # 4 token tiles
G = TT // P                 # 4 groups of 128 tokens per tile
NG = T_LOC // P             # 16 groups per core
QJ = 4                      # chunks per DMA quad
NQ = NCH // QJ              # 8 quads
N_WARM = 16                 # PE warm-up matmuls

BF16 = ml_dtypes.bfloat16

_CACHE = {}


def _pin_act_tables():
    """Make every ACT table set except natural_log_exp_and_others look like
    it contains none of {Copy, Exp, Ln}, so the table chooser emits a single
    ACT_TABLE_LOAD instead of thrashing between the exp and ln sets.
    Dict order/length is preserved (index == act_func_set_id)."""
    import concourse.hw_specs as hw_specs
    import concourse.bacc as bacc_mod
    import concourse.mybir as mybir

    if _CACHE.get("act_patched"):
        return
    orig = hw_specs.get_activation_tables
    mine = {mybir.ActivationFunctionType.Copy,
            mybir.ActivationFunctionType.Exp,
            mybir.ActivationFunctionType.Ln}

    def patched(module_arch):
        tables = dict(orig(module_arch))
        out = {}
        for name, funcs in tables.items():
            if name == "natural_log_exp_and_others":
                out[name] = funcs
            else:
                out[name] = funcs - mine
        return out

    hw_specs.get_activation_tables = patched
    bacc_mod.get_activation_tables = patched
    _CACHE["act_patched"] = True


def build_nc(t_loc=T_LOC):
    """Build the single-core Bass graph (same NEFF runs SPMD on all 8 cores)."""
    import os
    import concourse.bass as bass
    import concourse.bacc as bacc
    import concourse.mybir as mybir
    from concourse import tile
    from concourse.masks import make_identity
    from concourse.tile_rust import add_dep_helper

    stage = os.environ.get("K_STAGE", "full")   # mm | tr | sm | full
    nt_build = int(os.environ.get("K_NT", "0"))  # 0 = all tiles
    _pin_act_tables()

    nt = t_loc // TT
    ng = t_loc // P

    f32 = mybir.dt.float32
    bf16 = mybir.dt.bfloat16
    i32 = mybir.dt.int32
    u32 = mybir.dt.uint32
    AX = mybir.AxisListType.X
    OP = mybir.AluOpType
    AF = mybir.ActivationFunctionType

    nc = bacc.Bacc("TRN2", target_bir_lowering=False, debug=False)

    # ht[t, q, p, j*TT + u] = hiddenT[(4q+j)*128 + p, t*TT + u]
    ht = nc.declare_dram_parameter("ht", [nt, NQ, P, QJ * TT], bf16,
                                   isOutput=False)
    wt = nc.declare_dram_parameter("wt", [P, NCH * E], bf16, isOutput=False)
    oi = nc.declare_dram_parameter("oi", [P, ng, 4], i32, isOutput=True)
    ow = nc.declare_dram_parameter("ow", [P, ng, 4], bf16, isOutput=True)
    ok = nc.declare_dram_parameter("ok", [P, ng], i32, isOutput=True)

    with tile.TileContext(nc) as tc, ExitStack() as ctx:
        const = ctx.enter_context(tc.tile_pool(name="const", bufs=1))
        htp = ctx.enter_context(tc.tile_pool(name="htp", bufs=6))
        mmps = ctx.enter_context(
            tc.tile_pool(name="mmps", bufs=4, space=bass.MemorySpace.PSUM))
        mgps = ctx.enter_context(
            tc.tile_pool(name="mgps", bufs=2, space=bass.MemorySpace.PSUM))
        trps = ctx.enter_context(
            tc.tile_pool(name="trps", bufs=2, space=bass.MemorySpace.PSUM))
        work = ctx.enter_context(tc.tile_pool(name="work", bufs=2))
        outp = ctx.enter_context(tc.tile_pool(name="outp", bufs=1))

        wt_s = const.tile([P, NCH * E], bf16)
        nc.scalar.dma_start(wt_s[:], wt.ap())
        identb = const.tile([E, E], bf16)
        make_identity(nc, identb[:])
        identf = const.tile([P, P], f32)
        make_identity(nc, identf[:])
        eps = const.tile([P, 1], f32)
        nc.vector.memset(eps[:], 1e-9)
        oi_s = outp.tile([P, ng, 4], i32)
        ow_s = outp.tile([P, ng, 4], bf16)
        ok_s = outp.tile([P, ng], i32)
        if stage != "full":
            nc.gpsimd.memset(oi_s[:], 0)
            nc.gpsimd.memset(ow_s[:], 0.0)
            nc.gpsimd.memset(ok_s[:], 0)

        def emit_mm(t):
            # ---- router matmuls: even chunks in PE columns 0-63, odd in
            # 64-127 (col tiling) — two matmuls in flight, halving PE time
            # and letting LDWEIGHTS overlap. Half-sums land on PSUM
            # partitions 0-63 / 64-127.
            lg_ps = mmps.tile([P, TT], f32, tag="lgps")
            mid_mm = None
            for q in range(NQ):
                htile = htp.tile([P, QJ, TT], bf16, tag="ht")
                nc.sync.dma_start(htile[:], ht.ap()[t, q])
                for j in range(QJ):
                    c = QJ * q + j
                    half = c % 2
                    # skip_group_check: the sim's PSUM-group tracker is not
                    # partition-base aware; the two halves are physically
                    # disjoint partition ranges of the bank.
                    mm = nc.tensor.matmul(
                        lg_ps[half * E:(half + 1) * E, :],
                        wt_s[:, bass.ts(c, E)], htile[:, j, :],
                        start=(c < 2), stop=(c >= NCH - 2),
                        tile_position=(0, half * E),
                        skip_group_check=True)
                    if q == NQ // 2 and j == 0:
                        mid_mm = mm
            return lg_ps, mid_mm

        def emit_post(t, lg_ps, anchor):
            # ---- merge the two half-sums: copy PSUM out, then a cross-
            # partition SBUF->SBUF accumulate-DMA adds partitions 64-127
            # onto 0-63 in exact fp32.
            sb_all = work.tile([P, TT], f32, tag="sball")
            nc.scalar.copy(sb_all[:], lg_ps[:])
            nc.gpsimd.dma_start(sb_all[0:E, :], sb_all[E:P, :],
                                accum_op=OP.add)
            # single fp32 -> bf16 rounding (the reference's einsum output)
            lgb = work.tile([E, TT], bf16, tag="lgb")
            nc.scalar.copy(lgb[:], sb_all[0:E, :])

            # ---- PE transpose to token-major: (128 tok, G, 64 exp)
            tr_ps = trps.tile([P, G, E], bf16, tag="trps")
            for g in range(G):
                tr = nc.tensor.transpose(tr_ps[:, g, :], lgb[:, bass.ts(g, P)],
                                         identb[:])
                if anchor is not None:
                    # scheduling-only: keep this tile's transposes after the
                    # NEXT tile's final matmul so the PE never head-of-line
                    # stalls on the (slow) SWDGE merge chain
                    add_dep_helper(tr.ins, anchor.ins, sync=False,
                                   reason="tr after next tile's matmuls")
            lgt = work.tile([P, G, E], bf16, tag="lgt")
            nc.vector.tensor_copy(lgt[:], tr_ps[:])
            if stage == "tr":
                return

            # ---- softmax (fp32)
            mneg = work.tile([P, G], f32, tag="mneg")
            nc.vector.reduce_max(mneg[:], lgt[:], axis=AX, negate=True)
            pe_t = work.tile([P, G, E], f32, tag="pe")
            zs = work.tile([P, G], f32, tag="zs")
            for g in range(G):
                nc.scalar.activation(
                    pe_t[:, g, :], lgt[:, g, :], AF.Exp,
                    bias=mneg[:, g:g + 1], scale=1.0,
                    accum_out=zs[:, g:g + 1])
            rz = work.tile([P, G, 1], f32, tag="rz")
            nc.vector.reciprocal(rz[:, :, 0], zs[:])
            pn = work.tile([P, G, E], f32, tag="pn")
            a_bc, b_bc = bass.broadcast_tensor_aps(pe_t[:], rz[:])
            nc.vector.tensor_tensor(pn[:], a_bc, b_bc, OP.mult)

            # ---- entropy_neg = sum p*log(p + 1e-9)
            lq = work.tile([P, G, E], f32, tag="lq")
            nc.scalar.activation(lq[:], pn[:], AF.Ln, bias=eps[:], scale=1.0)
            pl = work.tile([P, G, E], f32, tag="pl")
            nc.vector.tensor_tensor(pl[:], pn[:], lq[:], OP.mult)
            entn = work.tile([P, G], f32, tag="entn")
            nc.vector.reduce_sum(entn[:], pl[:], axis=AX)

            # ---- adaptive k:  entropy<0.3 -> 1, >1.5 -> 4, else 2
            # entn = -entropy:  k>=2 iff entn <= -0.3 ;  k==4 iff entn < -1.5
            m2 = work.tile([P, G, 1], f32, tag="m2")
            nc.vector.tensor_scalar(m2[:, :, 0], entn[:], -0.3, None, OP.is_le)
            m4 = work.tile([P, G, 1], f32, tag="m4")
            nc.vector.tensor_scalar(m4[:, :, 0], entn[:], -1.5, None, OP.is_lt)
            kf = work.tile([P, G], f32, tag="kf")
            nc.vector.scalar_tensor_tensor(
                kf[:], m4[:, :, 0], 2.0, m2[:, :, 0], OP.mult, OP.add)
            nc.vector.tensor_scalar_add(kf[:], kf[:], 1.0)
            nc.vector.tensor_copy(ok_s[:, bass.ts(t, G)], kf[:])

            if stage == "sm":
                return
            # ---- top-8 values + indices (ties: descending value, ascending idx)
            mv = work.tile([P, G, 8], f32, tag="mv")
            mi = work.tile([P, G, 8], u32, tag="mi")
            for g in range(G):
                nc.vector.max(mv[:, g, :], pe_t[:, g, :])
                nc.vector.max_index(mi[:, g, :], mv[:, g, :], pe_t[:, g, :])

            # ---- active-slot mask (slot0: always, slot1: k>=2, slot2/3: k==4)
            act = work.tile([P, G, 4], f32, tag="act")
            nc.vector.memset(act[:, :, 0:1], 1.0)
            nc.vector.tensor_copy(act[:, :, 1:2], m2[:])
            nc.vector.tensor_copy(act[:, :, 2:3], m4[:])
            nc.vector.tensor_copy(act[:, :, 3:4], m4[:])

            # ---- masked renormalized weights
            w4 = work.tile([P, G, 4], f32, tag="w4")
            nc.vector.tensor_tensor(w4[:], mv[:, :, 0:4], act[:], OP.mult)
            ws = work.tile([P, G], f32, tag="ws")
            nc.vector.reduce_sum(ws[:], w4[:], axis=AX)
            rw = work.tile([P, G, 1], f32, tag="rw")
            nc.vector.reciprocal(rw[:, :, 0], ws[:])
            wn = work.tile([P, G, 4], f32, tag="wn")
            wa_bc, wb_bc = bass.broadcast_tensor_aps(w4[:], rw[:])
            nc.vector.tensor_tensor(wn[:], wa_bc, wb_bc, OP.mult)
            nc.vector.tensor_copy(ow_s[:, bass.ts(t, G)], wn[:])

            # ---- indices: (idx+1)*active - 1  (pads inactive slots with -1)
            idxf = work.tile([P, G, 4], f32, tag="idxf")
            nc.vector.tensor_copy(idxf[:], mi[:, :, 0:4])
            nc.vector.scalar_tensor_tensor(
                idxf[:], idxf[:], 1.0, act[:], OP.add, OP.mult)
            nc.vector.tensor_scalar_add(idxf[:], idxf[:], -1.0)
            nc.vector.tensor_copy(oi_s[:, bass.ts(t, G)], idxf[:])

        # software pipeline: tile t's matmuls stream while tile t-1's
        # transposes/softmax drain — only the last tile's post is a tail
        ntl = nt_build or nt
        prev = None
        for t in range(ntl):
            lg, mm = emit_mm(t)
            if prev is not None:
                emit_post(t - 1, prev, mm)
            prev = lg
        emit_post(ntl - 1, prev, None)

        nc.scalar.dma_start(oi.ap(), oi_s[:])
        nc.scalar.dma_start(ow.ap(), ow_s[:])
        nc.scalar.dma_start(ok.ap(), ok_s[:])

    nc.finalize()
    return nc


def _get_nc():
    if "nc" not in _CACHE:
        _CACHE["nc"] = build_nc()
    return _CACHE["nc"]


def _prep_shards(hidden, weight):
    hidden = np.asarray(hidden)
    weight = np.asarray(weight)
    if hidden.dtype != BF16:
        hidden = hidden.astype(BF16)
    if weight.dtype != BF16:
        weight = weight.astype(BF16)
    # weight (E, H) -> wt[p, c*E + e] = weight[e, c*P + p]
    wt = np.ascontiguousarray(
        weight.reshape(E, NCH, P).transpose(2, 1, 0).reshape(P, NCH * E))
    # hidden (T, H) -> hiddenT (H, T), shard along tokens, pack per-DMA-dense:
    # ht[t, q, p, j*TT+u] = hiddenT[(QJ*q+j)*P + p, t*TT + u]
    ht_full = np.ascontiguousarray(hidden.T)
    in_maps = []
    for c in range(N_CORES):
        s = ht_full[:, c * T_LOC:(c + 1) * T_LOC]          # (H, T_LOC)
        s5 = s.reshape(NQ, QJ, P, NT, TT)                  # (q, j, p, t, u)
        ht_shard = np.ascontiguousarray(
            s5.transpose(3, 0, 2, 1, 4).reshape(NT, NQ, P, QJ * TT))
        in_maps.append({"ht": ht_shard, "wt": wt})
    return in_maps


def _assemble(results):
    idx_parts, w_parts, k_parts = [], [], []
    for c in range(N_CORES):
        oi = np.asarray(results[c]["oi"])            # (P, NG, 4) int32
        ow = np.asarray(results[c]["ow"])            # (P, NG, 4) bf16
        ok = np.asarray(results[c]["ok"])            # (P, NG)   int32
        # token = g*128 + p  ->  [g, p, s]
        idx_parts.append(oi.transpose(1, 0, 2).reshape(T_LOC, 4))
        w_parts.append(ow.transpose(1, 0, 2).reshape(T_LOC, 4))
        k_parts.append(ok.transpose(1, 0).reshape(T_LOC))
    indices = np.concatenate(idx_parts, axis=0).astype(np.int32)
    weights = np.concatenate(w_parts, axis=0)
    if weights.dtype != BF16:
        weights = weights.view(BF16) if weights.dtype.itemsize == 2 \
            else weights.astype(BF16)
    k = np.concatenate(k_parts, axis=0).astype(np.int32)
    return indices, weights, k


def kernel(hidden, weight):
    from concourse.bass_utils import run_bass_kernel_spmd

    nc = _get_nc()
    in_maps = _prep_shards(hidden, weight)
    res = run_bass_kernel_spmd(nc, in_maps, core_ids=list(range(N_CORES)))
    return _assemble(res.results)


# revision 34
# speedup vs baseline: 1.2570x; 1.0139x over previous
"""Adaptive top-k MoE router on 8 TRN2 NeuronCores.

Data-parallel over tokens: each core routes T/8 = 2048 tokens.
Per core: weight-stationary bf16 matmul (fp32 PSUM accum) -> bf16 logits
-> PE transpose to token-major -> ACT exp/ln + DVE softmax/entropy/top-8
-> adaptive-k mask + renormalize -> int32/bf16 outputs.

Host side: shards hidden along T, pre-packed into a (t_tile, chunk_quad,
partition, contiguous-4KB) layout so every device DMA is a dense 512KB
transfer; replicates a rearranged weight; reassembles the full outputs.
"""

import numpy as np
import ml_dtypes
from contextlib import ExitStack

P = 128            # SBUF partitions
E = 64             # experts
H = 4096           # hidden dim
T_FULL = 16384     # total tokens
N_CORES = 8
T_LOC = T_FULL // N_CORES   # 2048 tokens per core
NCH = H // P                # 32 contraction chunks
TT = 512                    # token tile (matmul free dim)
NT = T_LOC // TT            # 4 token tiles
G = TT // P                 # 4 groups of 128 tokens per tile
NG = T_LOC // P             # 16 groups per core
QJ = 4                      # chunks per DMA quad
NQ = NCH // QJ              # 8 quads
N_WARM = 16                 # PE warm-up matmuls

BF16 = ml_dtypes.bfloat16

_CACHE = {}


def _pin_act_tables():
    """Make every ACT table set except natural_log_exp_and_others look like
    it contains none of {Copy, Exp, Ln}, so the table chooser emits a single
    ACT_TABLE_LOAD instead of thrashing between the exp and ln sets.
    Dict order/length is preserved (index == act_func_set_id)."""
    import concourse.hw_specs as hw_specs
    import concourse.bacc as bacc_mod
    import concourse.mybir as mybir

    if _CACHE.get("act_patched"):
        return
    orig = hw_specs.get_activation_tables
    mine = {mybir.ActivationFunctionType.Copy,
            mybir.ActivationFunctionType.Exp,
            mybir.ActivationFunctionType.Ln}

    def patched(module_arch):
        tables = dict(orig(module_arch))
        out = {}
        for name, funcs in tables.items():
            if name == "natural_log_exp_and_others":
                out[name] = funcs
            else:
                out[name] = funcs - mine
        return out

    hw_specs.get_activation_tables = patched
    bacc_mod.get_activation_tables = patched
    _CACHE["act_patched"] = True


def build_nc(t_loc=T_LOC):
    """Build the single-core Bass graph (same NEFF runs SPMD on all 8 cores)."""
    import os
    import concourse.bass as bass
    import concourse.bacc as bacc
    import concourse.mybir as mybir
    from concourse import tile
    from concourse.masks import make_identity
    from concourse.tile_rust import add_dep_helper

    stage = os.environ.get("K_STAGE", "full")   # mm | tr | sm | full
    nt_build = int(os.environ.get("K_NT", "0"))  # 0 = all tiles
    _pin_act_tables()

    nt = t_loc // TT
    ng = t_loc // P

    f32 = mybir.dt.float32
    bf16 = mybir.dt.bfloat16
    i32 = mybir.dt.int32
    u32 = mybir.dt.uint32
    AX = mybir.AxisListType.X
    OP = mybir.AluOpType
    AF = mybir.ActivationFunctionType

    nc = bacc.Bacc("TRN2", target_bir_lowering=False, debug=False)

    # ht[t, q, p, j*TT + u] = hiddenT[(4q+j)*128 + p, t*TT + u]
    ht = nc.declare_dram_parameter("ht", [nt, NQ, P, QJ * TT], bf16,
                                   isOutput=False)
    wt = nc.declare_dram_parameter("wt", [P, NCH * E], bf16, isOutput=False)
    oi = nc.declare_dram_parameter("oi", [P, ng, 4], i32, isOutput=True)
    ow = nc.declare_dram_parameter("ow", [P, ng, 4], bf16, isOutput=True)
    ok = nc.declare_dram_parameter("ok", [P, ng], i32, isOutput=True)

    with tile.TileContext(nc) as tc, ExitStack() as ctx:
        const = ctx.enter_context(tc.tile_pool(name="const", bufs=1))
        htp = ctx.enter_context(tc.tile_pool(name="htp", bufs=8))
        mmps = ctx.enter_context(
            tc.tile_pool(name="mmps", bufs=4, space=bass.MemorySpace.PSUM))
        mgps = ctx.enter_context(
            tc.tile_pool(name="mgps", bufs=2, space=bass.MemorySpace.PSUM))
        trps = ctx.enter_context(
            tc.tile_pool(name="trps", bufs=2, space=bass.MemorySpace.PSUM))
        work = ctx.enter_context(tc.tile_pool(name="work", bufs=2))
        outp = ctx.enter_context(tc.tile_pool(name="outp", bufs=1))

        wt_s = const.tile([P, NCH * E], bf16)
        nc.scalar.dma_start(wt_s[:], wt.ap())
        identb = const.tile([E, E], bf16)
        make_identity(nc, identb[:])
        eps = const.tile([P, 1], f32)
        nc.vector.memset(eps[:], 1e-9)
        oi_s = outp.tile([P, ng, 4], i32)
        ow_s = outp.tile([P, ng, 4], bf16)
        ok_s = outp.tile([P, ng], i32)
        if stage != "full":
            nc.gpsimd.memset(oi_s[:], 0)
            nc.gpsimd.memset(ow_s[:], 0.0)
            nc.gpsimd.memset(ok_s[:], 0)

        def emit_mm(t):
            # ---- router matmuls: even chunks in PE columns 0-63, odd in
            # 64-127 (col tiling) — two matmuls in flight, halving PE time
            # and letting LDWEIGHTS overlap. Half-sums land on PSUM
            # partitions 0-63 / 64-127.
            lg_ps = mmps.tile([P, TT], f32, tag="lgps")
            mid_mm = None
            for q in range(NQ):
                htile = htp.tile([P, QJ, TT], bf16, tag="ht")
                nc.sync.dma_start(htile[:], ht.ap()[t, q])
                for j in range(QJ):
                    c = QJ * q + j
                    half = c % 2
                    # skip_group_check: the sim's PSUM-group tracker is not
                    # partition-base aware; the two halves are physically
                    # disjoint partition ranges of the bank.
                    mm = nc.tensor.matmul(
                        lg_ps[half * E:(half + 1) * E, :],
                        wt_s[:, bass.ts(c, E)], htile[:, j, :],
                        start=(c < 2), stop=(c >= NCH - 2),
                        tile_position=(0, half * E),
                        skip_group_check=True)
                    if q == NQ // 2 and j == 0:
                        mid_mm = mm
            return lg_ps, mid_mm

        def emit_post(t, lg_ps, anchor):
            # ---- merge the two half-sums: copy PSUM out, then a cross-
            # partition SBUF->SBUF accumulate-DMA adds partitions 64-127
            # onto 0-63 in exact fp32.
            sb_all = work.tile([P, TT], f32, tag="sball")
            nc.scalar.copy(sb_all[:], lg_ps[:])
            nc.gpsimd.dma_start(sb_all[0:E, :], sb_all[E:P, :],
                                accum_op=OP.add)
            # single fp32 -> bf16 rounding (the reference's einsum output)
            lgb = work.tile([E, TT], bf16, tag="lgb")
            nc.scalar.copy(lgb[:], sb_all[0:E, :])

            # ---- PE transpose to token-major: (128 tok, G, 64 exp)
            tr_ps = trps.tile([P, G, E], bf16, tag="trps")
            for g in range(G):
                tr = nc.tensor.transpose(tr_ps[:, g, :], lgb[:, bass.ts(g, P)],
                                         identb[:])
                if anchor is not None:
                    # scheduling-only: keep this tile's transposes after the
                    # NEXT tile's final matmul so the PE never head-of-line
                    # stalls on the (slow) SWDGE merge chain
                    add_dep_helper(tr.ins, anchor.ins, sync=False,
                                   reason="tr after next tile's matmuls")
            lgt = work.tile([P, G, E], bf16, tag="lgt")
            nc.vector.tensor_copy(lgt[:], tr_ps[:])
            if stage == "tr":
                return

            # ---- softmax (fp32)
            mneg = work.tile([P, G], f32, tag="mneg")
            nc.vector.reduce_max(mneg[:], lgt[:], axis=AX, negate=True)
            pe_t = work.tile([P, G, E], f32, tag="pe")
            zs = work.tile([P, G], f32, tag="zs")
            for g in range(G):
                nc.scalar.activation(
                    pe_t[:, g, :], lgt[:, g, :], AF.Exp,
                    bias=mneg[:, g:g + 1], scale=1.0,
                    accum_out=zs[:, g:g + 1])
            rz = work.tile([P, G, 1], f32, tag="rz")
            nc.vector.reciprocal(rz[:, :, 0], zs[:])
            pn = work.tile([P, G, E], f32, tag="pn")
            a_bc, b_bc = bass.broadcast_tensor_aps(pe_t[:], rz[:])
            nc.vector.tensor_tensor(pn[:], a_bc, b_bc, OP.mult)

            # ---- entropy_neg = sum p*log(p + 1e-9)
            lq = work.tile([P, G, E], f32, tag="lq")
            nc.scalar.activation(lq[:], pn[:], AF.Ln, bias=eps[:], scale=1.0)
            pl = work.tile([P, G, E], f32, tag="pl")
            nc.vector.tensor_tensor(pl[:], pn[:], lq[:], OP.mult)
            entn = work.tile([P, G], f32, tag="entn")
            nc.vector.reduce_sum(entn[:], pl[:], axis=AX)

            # ---- adaptive k:  entropy<0.3 -> 1, >1.5 -> 4, else 2
            # entn = -entropy:  k>=2 iff entn <= -0.3 ;  k==4 iff entn < -1.5
            m2 = work.tile([P, G, 1], f32, tag="m2")
            nc.vector.tensor_scalar(m2[:, :, 0], entn[:], -0.3, None, OP.is_le)
            m4 = work.tile([P, G, 1], f32, tag="m4")
            nc.vector.tensor_scalar(m4[:, :, 0], entn[:], -1.5, None, OP.is_lt)
            kf = work.tile([P, G], f32, tag="kf")
            nc.vector.scalar_tensor_tensor(
                kf[:], m4[:, :, 0], 2.0, m2[:, :, 0], OP.mult, OP.add)
            nc.vector.tensor_scalar_add(kf[:], kf[:], 1.0)
            nc.vector.tensor_copy(ok_s[:, bass.ts(t, G)], kf[:])

            if stage == "sm":
                return
            # ---- top-8 values + indices (ties: descending value, ascending idx)
            mv = work.tile([P, G, 8], f32, tag="mv")
            mi = work.tile([P, G, 8], u32, tag="mi")
            for g in range(G):
                nc.vector.max(mv[:, g, :], pe_t[:, g, :])
                nc.vector.max_index(mi[:, g, :], mv[:, g, :], pe_t[:, g, :])

            # ---- active-slot mask (slot0: always, slot1: k>=2, slot2/3: k==4)
            act = work.tile([P, G, 4], f32, tag="act")
            nc.vector.memset(act[:, :, 0:1], 1.0)
            nc.vector.tensor_copy(act[:, :, 1:2], m2[:])
            nc.vector.tensor_copy(act[:, :, 2:3], m4[:])
            nc.vector.tensor_copy(act[:, :, 3:4], m4[:])

            # ---- masked renormalized weights
            w4 = work.tile([P, G, 4], f32, tag="w4")
            nc.vector.tensor_tensor(w4[:], mv[:, :, 0:4], act[:], OP.mult)
            ws = work.tile([P, G], f32, tag="ws")
            nc.vector.reduce_sum(ws[:], w4[:], axis=AX)
            rw = work.tile([P, G, 1], f32, tag="rw")
            nc.vector.reciprocal(rw[:, :, 0], ws[:])
            wn = work.tile([P, G, 4], f32, tag="wn")
            wa_bc, wb_bc = bass.broadcast_tensor_aps(w4[:], rw[:])
            nc.vector.tensor_tensor(wn[:], wa_bc, wb_bc, OP.mult)
            nc.vector.tensor_copy(ow_s[:, bass.ts(t, G)], wn[:])

            # ---- indices: (idx+1)*active - 1  (pads inactive slots with -1)
            idxf = work.tile([P, G, 4], f32, tag="idxf")
            nc.vector.tensor_copy(idxf[:], mi[:, :, 0:4])
            nc.vector.scalar_tensor_tensor(
                idxf[:], idxf[:], 1.0, act[:], OP.add, OP.mult)
            nc.vector.tensor_scalar_add(idxf[:], idxf[:], -1.0)
            nc.vector.tensor_copy(oi_s[:, bass.ts(t, G)], idxf[:])

        # software pipeline: tile t's matmuls stream while tile t-1's
        # transposes/softmax drain — only the last tile's post is a tail
        ntl = nt_build or nt
        prev = None
        for t in range(ntl):
            lg, mm = emit_mm(t)
            if prev is not None:
                emit_post(t - 1, prev, mm)
            prev = lg
        emit_post(ntl - 1, prev, None)

        nc.scalar.dma_start(oi.ap(), oi_s[:])
        nc.scalar.dma_start(ow.ap(), ow_s[:])
        nc.scalar.dma_start(ok.ap(), ok_s[:])

    nc.finalize()
    return nc


def _get_nc():
    if "nc" not in _CACHE:
        _CACHE["nc"] = build_nc()
    return _CACHE["nc"]


def _prep_shards(hidden, weight):
    hidden = np.asarray(hidden)
    weight = np.asarray(weight)
    if hidden.dtype != BF16:
        hidden = hidden.astype(BF16)
    if weight.dtype != BF16:
        weight = weight.astype(BF16)
    # weight (E, H) -> wt[p, c*E + e] = weight[e, c*P + p]
    wt = np.ascontiguousarray(
        weight.reshape(E, NCH, P).transpose(2, 1, 0).reshape(P, NCH * E))
    # hidden (T, H) -> hiddenT (H, T), shard along tokens, pack per-DMA-dense:
    # ht[t, q, p, j*TT+u] = hiddenT[(QJ*q+j)*P + p, t*TT + u]
    ht_full = np.ascontiguousarray(hidden.T)
    in_maps = []
    for c in range(N_CORES):
        s = ht_full[:, c * T_LOC:(c + 1) * T_LOC]          # (H, T_LOC)
        s5 = s.reshape(NQ, QJ, P, NT, TT)                  # (q, j, p, t, u)
        ht_shard = np.ascontiguousarray(
            s5.transpose(3, 0, 2, 1, 4).reshape(NT, NQ, P, QJ * TT))
        in_maps.append({"ht": ht_shard, "wt": wt})
    return in_maps


def _assemble(results):
    idx_parts, w_parts, k_parts = [], [], []
    for c in range(N_CORES):
        oi = np.asarray(results[c]["oi"])            # (P, NG, 4) int32
        ow = np.asarray(results[c]["ow"])            # (P, NG, 4) bf16
        ok = np.asarray(results[c]["ok"])            # (P, NG)   int32
        # token = g*128 + p  ->  [g, p, s]
        idx_parts.append(oi.transpose(1, 0, 2).reshape(T_LOC, 4))
        w_parts.append(ow.transpose(1, 0, 2).reshape(T_LOC, 4))
        k_parts.append(ok.transpose(1, 0).reshape(T_LOC))
    indices = np.concatenate(idx_parts, axis=0).astype(np.int32)
    weights = np.concatenate(w_parts, axis=0)
    if weights.dtype != BF16:
        weights = weights.view(BF16) if weights.dtype.itemsize == 2 \
            else weights.astype(BF16)
    k = np.concatenate(k_parts, axis=0).astype(np.int32)
    return indices, weights, k


def kernel(hidden, weight):
    from concourse.bass_utils import run_bass_kernel_spmd

    nc = _get_nc()
    in_maps = _prep_shards(hidden, weight)
    res = run_bass_kernel_spmd(nc, in_maps, core_ids=list(range(N_CORES)))
    return _assemble(res.results)
